# revision 19
# baseline (speedup 1.0000x reference)
"""Trainium2 Bass kernel for nn_CellularTransformer.

Sharding: data-parallel over the B=8 complexes (one complex per NeuronCore).
belong* arrays are sorted, so each complex occupies a contiguous row range in
every cell dimension; the block-diagonal batch mask zeroes all cross-complex
attention, so each core only needs the diagonal blocks of the adjacency /
boundary matrices. Small weights are replicated.

Per-core padded sizes (multiples of 128 for partition chunking):
  dim0: 256 cells (actual max block 210), dim1: 512 (402), dim2: 256 (213).
Target-side free dims are 256/416/256 (>=256 keeps fp32r matmuls at full rate).

Head layout: PE operand/output partition bases must be in {0,32,64}, so the
8 heads live in 3 "slots" of a [128, 3, *] tile, 3 heads per slot at bases
0/32/64 (slot 2 holds 2 heads + a phantom filler).  QK^T is computed per head
in [source, target] orientation; exp (no max subtraction — scores are O(10))
runs on ScalarE into bf16; the 0/1 mask multiply runs on VectorE in bf16; the
AV matmul uses a 32-wide augmented V (8 v-cols, a ones column producing the
softmax denominator D at row 32*h3+8, zero padding) so each 32-row PSUM group
is fully written.  The last padded source row of every mask is 1e-30, so D is
never exactly 0 (no NaNs, and rows with no neighbors come out exactly 0 after
the mask, matching the reference's softmax(-1e30)*mask behavior).
"""

import numpy as np
import ml_dtypes

import concourse.bass as bass
import concourse.mybir as mybir
import concourse.tile as tile
from concourse.bass_utils import run_bass_kernel_spmd
from concourse.masks import make_identity

F32 = mybir.dt.float32
F32R = mybir.dt.float32r
BF16 = mybir.dt.bfloat16
AF = mybir.ActivationFunctionType
ALU = mybir.AluOpType

NCORES = 8
L, NH, DH, H, OUT = 2, 8, 8, 64, 10
FDIM, PDIM = 32, 16
INTER = [(0, 0), (0, 1), (1, 0), (1, 1), (1, 2), (2, 1), (2, 2)]
NCH = [2, 4, 2]                 # 128-partition source chunks per dim
NPC = [c * 128 for c in NCH]    # padded cell count (source side)
NPF = [256, 416, 256]           # padded cell count (target/free side)
SCALE = 1.0 / float(np.sqrt(DH))
TD_FIRST = {0: 0, 1: 1, 2: 4}
TD_LAST = {0: 2, 1: 5, 2: 6}
PHANTOM_EPS = 1e-30

_PROGRAM_CACHE = {}
last_results = None  # test.py introspection (exec time / profile)


def _build_program():
    nc = bass.Bass("TRN2", target_bir_lowering=False, debug=False, num_devices=NCORES)

    dt_in = {}

    def din(name, shape, dtype=F32):
        dt_in[name] = nc.dram_tensor(name, list(shape), dtype, kind="ExternalInput")

    for d in range(3):
        din(f"xT{d}", (FDIM, NPC[d]), F32R)
        din(f"peT{d}", (PDIM, NPC[d]), F32R)
        din(f"vm{d}", (128, NCH[d]))
    for ii, (sd, td) in enumerate(INTER):
        din(f"mT{ii}", (128, NCH[sd], NPF[td]), BF16)
    for d in range(3):
        din(f"Wf{d}", (FDIM, H), F32R)
        din(f"Wp{d}", (PDIM, H), F32R)
    din("WqP", (H, L * 7, 3, 128), F32R)
    din("WkP", (H, L * 7, 3, 128), F32R)
    din("WvR", (H, L * 7, H), F32R)
    din("WoP", (128, L * 7, 2, H), F32R)
    din("Wff1R", (H, L * 3, 4 * H), F32R)
    din("Wff2R", (128, L * 3 * 2, H), F32R)
    din("Wh1", (H, H))
    din("Wh2", (H, H))
    din("Wh3", (H, OUT))
    y_out = nc.dram_tensor("y", [1, OUT], F32, kind="ExternalOutput")

    from contextlib import ExitStack
    with tile.TileContext(nc) as tc, ExitStack() as es:
        const = es.enter_context(tc.tile_pool(name="const", bufs=1))
        state = es.enter_context(tc.tile_pool(name="state", bufs=1))
        work = es.enter_context(tc.tile_pool(name="work", bufs=3))
        pp_sc = es.enter_context(tc.tile_pool(name="pp_sc", bufs=2, space="PSUM"))
        pp_av = es.enter_context(tc.tile_pool(name="pp_av", bufs=1, space="PSUM"))
        pp_acc = es.enter_context(tc.tile_pool(name="pp_acc", bufs=1, space="PSUM"))
        pp_wk = es.enter_context(tc.tile_pool(name="pp_wk", bufs=2, space="PSUM"))

        ident = const.tile([128, 128], F32, tag="ident", name="ident")
        make_identity(nc, ident[:])

        def load(name, shape, tag, dtype=F32):
            t = const.tile(list(shape), dtype, tag=tag, name=tag)
            nc.sync.dma_start(out=t[:], in_=dt_in[name].ap())
            return t

        xT = [load(f"xT{d}", (FDIM, NPC[d]), f"xT{d}", F32R) for d in range(3)]
        peT = [load(f"peT{d}", (PDIM, NPC[d]), f"peT{d}", F32R) for d in range(3)]
        vm = [load(f"vm{d}", (128, NCH[d]), f"vm{d}") for d in range(3)]
        mT = [load(f"mT{ii}", (128, NCH[sd], NPF[td]), f"mT{ii}", BF16)
              for ii, (sd, td) in enumerate(INTER)]
        Wf = [load(f"Wf{d}", (FDIM, H), f"Wf{d}", F32R) for d in range(3)]
        Wp = [load(f"Wp{d}", (PDIM, H), f"Wp{d}", F32R) for d in range(3)]
        WqP = load("WqP", (H, L * 7, 3, 128), "WqP", F32R)
        WkP = load("WkP", (H, L * 7, 3, 128), "WkP", F32R)
        WvR = load("WvR", (H, L * 7, H), "WvR", F32R)
        WoP = load("WoP", (128, L * 7, 2, H), "WoP", F32R)
        Wff1 = load("Wff1R", (H, L * 3, 4 * H), "Wff1R", F32R)
        Wff2 = load("Wff2R", (128, L * 3 * 2, H), "Wff2R", F32R)
        Wh1 = load("Wh1", (H, H), "Wh1")
        Wh2 = load("Wh2", (H, H), "Wh2")
        Wh3 = load("Wh3", (H, OUT), "Wh3")

        # persistent state
        hT = [state.tile([H, NPC[d]], F32R, tag=f"hT{d}", name=f"hT{d}") for d in range(3)]
        h_ = [state.tile([128, NCH[d], H], F32, tag=f"h{d}", name=f"h{d}") for d in range(3)]
        h1 = [state.tile([128, NCH[d], H], F32, tag=f"h1{d}", name=f"h1{d}") for d in range(3)]
        hT1 = [state.tile([H, NPC[d]], F32R, tag=f"hT1{d}", name=f"hT1{d}") for d in range(3)]
        for d in range(3):
            nc.vector.memset(h_[d][:], 0.0)
            nc.vector.memset(h1[d][:], 0.0)
        # manually double-buffered augmented-V tiles: [128, chunk, head, 32]
        # col 8 = 1 (denominator source), cols 9..31 = 0 (full 32-row groups)
        vaug = [state.tile([128, 4, NH, 32], BF16, tag=f"vaug{j}", name=f"vaug{j}")
                for j in range(2)]
        vpat = state.tile([128, 32], F32, tag="vpat", name="vpat")
        nc.vector.memset(vpat[:], 0.0)
        nc.vector.memset(vpat[:, 8:9], 1.0)
        for j in range(2):
            nc.vector.tensor_copy(
                vaug[j][:],
                vpat[:].unsqueeze(1).unsqueeze(1).to_broadcast((128, 4, NH, 32)))
        eps = state.tile([128, 1], F32, tag="eps", name="eps")
        nc.vector.memset(eps[:], 1e-5)

        def r(ap):
            return ap.bitcast(F32R)

        # embed
        for d in range(3):
            ps = pp_wk.tile([128, 512], F32, tag="wk", name="wk")
            emb = ps[:H, : NPC[d]]
            nc.tensor.matmul(emb, r(Wf[d][:]), r(xT[d][:]), start=True, stop=False)
            nc.tensor.matmul(emb, r(Wp[d][:]), r(peT[d][:]), start=False, stop=True)
            nc.scalar.copy(out=hT[d][:], in_=emb)
            for c in range(NCH[d]):
                tp = pp_wk.tile([128, 512], F32, tag="wk", name="wk")
                nc.tensor.transpose(tp[:128, :H],
                                    hT[d][:, c * 128:(c + 1) * 128].bitcast(F32),
                                    ident[:H, :H])
                nc.scalar.copy(out=h_[d][:, c, :], in_=tp[:128, :H])

        shuf_mask = [8] * 32

        for l in range(L):
            acc02 = pp_acc.tile([H, 512], F32, tag="acc02", name="acc02")
            acc1 = pp_acc.tile([H, 512], F32, tag="acc1", name="acc1")
            accv = {0: acc02[:, 0:256], 1: acc1[:, 0:416], 2: acc02[:, 256:512]}
            for ii, (sd, td) in enumerate(INTER):
                li = l * 7 + ii
                T = NPF[td]
                nsc = NCH[sd]
                # qT/kT head-slot layout [128, 3, cells]; head hd -> slot
                # t3 = hd // 3 at partition base 32*(hd % 3)
                qT = work.tile([128, 3, 512], F32R, tag="qT", name="qT", bufs=2)
                kT = work.tile([128, 3, 512], F32R, tag="kT", name="kT", bufs=2)
                for t3 in range(3):
                    qps = pp_wk.tile([128, 512], F32, tag="wk", name="wk")
                    nc.tensor.matmul(qps[:, :T], r(WqP[:, li, t3, :]),
                                     r(hT[td][:, :T]), start=True, stop=True)
                    nc.scalar.copy(out=qT[:, t3, :T], in_=qps[:, :T])
                    kps = pp_wk.tile([128, 512], F32, tag="wk", name="wk")
                    nc.tensor.matmul(kps[:, : NPC[sd]], r(WkP[:, li, t3, :]),
                                     r(hT[sd][:]), start=True, stop=True)
                    nc.scalar.copy(out=kT[:, t3, : NPC[sd]], in_=kps[:, : NPC[sd]])
                vsb = vaug[ii % 2]
                for c in range(nsc):
                    vps = pp_wk.tile([128, 512], F32, tag="wk", name="wk")
                    nc.tensor.matmul(vps[:128, :H],
                                     r(hT[sd][:, c * 128:(c + 1) * 128]),
                                     r(WvR[:, li, :]), start=True, stop=True)
                    nc.scalar.copy(
                        out=vsb[:, c, :, 0:8],
                        in_=vps[:128, :H].rearrange("p (a b) -> p a b", a=NH))
                # attention: per head, single-bank score tiles (pipelined
                # with exp/mask), AV into a fully packed 2-slot psum: slot hf
                # holds heads 3*hf..3*hf+2 at bases 0/32/64 and head 6+hf at
                # base 96 (explicit tile_position, since implicit placement
                # rejects base 96).
                avp = pp_av.tile([128, 2, 512], F32, tag="av", name="av")
                for c in range(nsc):
                    for hd in range(NH):
                        t3, b3 = hd // 3, 32 * (hd % 3)
                        scp = pp_sc.tile([128, 512], F32, tag="sc", name="sc")
                        nc.tensor.matmul(
                            scp[:, :T],
                            r(kT[b3:b3 + 8, t3, c * 128:(c + 1) * 128]),
                            r(qT[b3:b3 + 8, t3, :T]),
                            start=True, stop=True)
                        et = work.tile([128, 512], BF16, tag="et", name="et")
                        nc.scalar.activation(out=et[:, :T], in_=scp[:, :T],
                                             func=AF.Exp, scale=SCALE)
                        pt = work.tile([128, 512], BF16, tag="pt", name="pt")
                        nc.vector.tensor_mul(pt[:, :T], et[:, :T], mT[ii][:, c, :])
                        if hd < 6:
                            hf, bo = hd // 3, 32 * (hd % 3)
                            tpos = None
                        else:
                            hf, bo = hd - 6, 96
                            tpos = (0, 96)
                        nc.tensor.matmul(
                            avp[bo:bo + 32, hf, :T],
                            vsb[:, c, hd, :],
                            pt[:, :T],
                            start=(c == 0), stop=(c == nsc - 1),
                            skip_group_check=True, tile_position=tpos)
                # normalize: o = av * shuffle(1/D); D at row base+8 of each
                # 32-group (phantom mask row keeps D >= 1e-30)
                rav = work.tile([128, 2, 512], F32, tag="rav", name="rav", bufs=2)
                nc.vector.stream_shuffle(out=rav[:, :, :T], in_=avp[:, :, :T],
                                         mask=shuf_mask)
                dbc = work.tile([128, 2, 512], F32, tag="dbc", name="dbc", bufs=2)
                nc.vector.reciprocal(out=dbc[:, :, :T], in_=rav[:, :, :T])
                onm = work.tile([128, 2, 512], F32R, tag="onm", name="onm", bufs=2)
                nc.vector.tensor_mul(onm[:, :, :T], avp[:, :, :T], dbc[:, :, :T])
                for hf in range(2):
                    nc.tensor.matmul(
                        accv[td], r(WoP[:, li, hf, :]),
                        r(onm[:, hf, :T]),
                        start=(ii == TD_FIRST[td] and hf == 0),
                        stop=(ii == TD_LAST[td] and hf == 1))

            # residual + LN1 + FFN + LN2 per dim
            for d in range(3):
                T = NPF[d]
                accs = work.tile([H, 512], F32, tag="accs", name="accs", bufs=2)
                nc.scalar.copy(out=accs[:, :T], in_=accv[d])
                for c in range((T + 127) // 128):
                    w = min(128, T - c * 128)
                    tp = pp_wk.tile([128, 512], F32, tag="wk", name="wk")
                    nc.tensor.transpose(tp[:w, :H], accs[:, c * 128:c * 128 + w],
                                        ident[:H, :H])
                    hs = work.tile([128, H], F32, tag="hs", name="hs")
                    nc.vector.tensor_add(hs[:w], h_[d][:w, c, :], tp[:w, :H])
                    st = work.tile([128, 6], F32, tag="st", name="st")
                    nc.vector.bn_stats(out=st[:w], in_=hs[:w])
                    mv = work.tile([128, 2], F32, tag="mv", name="mv")
                    nc.vector.bn_aggr(out=mv[:w], in_=st[:w])
                    sd_ = work.tile([128, 1], F32, tag="sd", name="sd")
                    nc.scalar.activation(out=sd_[:w], in_=mv[:w, 1:2], func=AF.Sqrt,
                                         bias=eps[:w])
                    nc.vector.reciprocal(out=sd_[:w], in_=sd_[:w])
                    nc.vector.tensor_scalar(out=h1[d][:w, c, :], in0=hs[:w],
                                            scalar1=mv[:w, 0:1], scalar2=sd_[:w],
                                            op0=ALU.subtract, op1=ALU.mult)
                    tp2 = pp_wk.tile([128, 512], F32, tag="wk", name="wk")
                    nc.tensor.transpose(tp2[:H, :w], h1[d][:w, c, :], ident[:w, :w])
                    nc.scalar.copy(out=hT1[d][:, c * 128:c * 128 + w], in_=tp2[:H, :w])
                f1 = work.tile([128, 2, 512], F32R, tag="f1", name="f1", bufs=2)
                for fc in range(2):
                    fps = pp_wk.tile([128, 512], F32, tag="wk", name="wk")
                    nc.tensor.matmul(fps[:, :T],
                                     r(Wff1[:, l * 3 + d, fc * 128:(fc + 1) * 128]),
                                     r(hT1[d][:, :T]), start=True, stop=True)
                    nc.scalar.activation(out=f1[:, fc, :T], in_=fps[:, :T],
                                         func=AF.Relu)
                for c in range((T + 127) // 128):
                    w = min(128, T - c * 128)
                    fp2 = pp_wk.tile([128, 512], F32, tag="wk", name="wk")
                    for fc in range(2):
                        nc.tensor.matmul(fp2[:w, :H],
                                         r(f1[:, fc, c * 128:c * 128 + w]),
                                         r(Wff2[:, (l * 3 + d) * 2 + fc, :]),
                                         start=(fc == 0), stop=(fc == 1))
                    hs2 = work.tile([128, H], F32, tag="hs2", name="hs2")
                    nc.vector.tensor_add(hs2[:w], h1[d][:w, c, :], fp2[:w, :H])
                    st2 = work.tile([128, 6], F32, tag="st2", name="st2")
                    nc.vector.bn_stats(out=st2[:w], in_=hs2[:w])
                    mv2 = work.tile([128, 2], F32, tag="mv2", name="mv2")
                    nc.vector.bn_aggr(out=mv2[:w], in_=st2[:w])
                    sd2 = work.tile([128, 1], F32, tag="sd2", name="sd2")
                    nc.scalar.activation(out=sd2[:w], in_=mv2[:w, 1:2], func=AF.Sqrt,
                                         bias=eps[:w])
                    nc.vector.reciprocal(out=sd2[:w], in_=sd2[:w])
                    nc.vector.tensor_scalar(out=h_[d][:w, c, :], in0=hs2[:w],
                                            scalar1=mv2[:w, 0:1], scalar2=sd2[:w],
                                            op0=ALU.subtract, op1=ALU.mult)
                    tp3 = pp_wk.tile([128, 512], F32, tag="wk", name="wk")
                    nc.tensor.transpose(tp3[:H, :w], h_[d][:w, c, :], ident[:w, :w])
                    nc.scalar.copy(out=hT[d][:, c * 128:c * 128 + w],
                                   in_=tp3[:H, :w])

        # pooling (masked mean; vm carries 1/count) + head MLP
        plp = pp_wk.tile([128, 512], F32, tag="wk", name="wk")
        first = True
        for d in range(3):
            for c in range(NCH[d]):
                last = (d == 2 and c == NCH[2] - 1)
                nc.tensor.matmul(plp[:1, :H], vm[d][:, c:c + 1], h_[d][:, c, :],
                                 start=first, stop=last)
                first = False
        pls = work.tile([1, H], F32, tag="pls", name="pls")
        nc.scalar.copy(out=pls[:], in_=plp[:1, :H])
        ptp = pp_wk.tile([128, 512], F32, tag="wk", name="wk")
        nc.tensor.transpose(ptp[:H, :1], pls[:], ident[:1, :1])
        plT = work.tile([H, 1], F32, tag="plT", name="plT")
        nc.scalar.copy(out=plT[:], in_=ptp[:H, :1])
        y1p = pp_wk.tile([128, 512], F32, tag="wk", name="wk")
        nc.tensor.matmul(y1p[:H, :1], Wh1[:], plT[:], start=True, stop=True)
        y1 = work.tile([H, 1], F32, tag="y1", name="y1")
        nc.scalar.activation(out=y1[:], in_=y1p[:H, :1], func=AF.Relu)
        y2p = pp_wk.tile([128, 512], F32, tag="wk", name="wk")
        nc.tensor.matmul(y2p[:H, :1], Wh2[:], y1[:], start=True, stop=True)
        y2 = work.tile([H, 1], F32, tag="y2", name="y2")
        nc.scalar.activation(out=y2[:], in_=y2p[:H, :1], func=AF.Relu)
        y3p = pp_wk.tile([128, 512], F32, tag="wk", name="wk")
        nc.tensor.matmul(y3p[:OUT, :1], Wh3[:], y2[:], start=True, stop=True)
        y3 = work.tile([OUT, 1], F32, tag="y3", name="y3")
        nc.scalar.copy(out=y3[:], in_=y3p[:OUT, :1])
        nc.sync.dma_start(out=y_out.ap().rearrange("a b -> b a"), in_=y3[:])

    return nc


_SPLIT_ENGINES = frozenset(("PE", "Activation", "DVE", "Pool", "SP"))


def _split_matmul_waits(nc):
    """TPB engine instructions carry at most one semaphore wait. Move all but
    one wait onto same-engine Drain instructions inserted just before."""
    k = 0
    for f in nc.m.functions:
        for bb in f.blocks:
            insts = bb.instructions
            out = []
            changed = False
            for i in insts:
                eng = getattr(i, "engine", None)
                if (eng is not None and eng.name in _SPLIT_ENGINES
                        and i.opcode != "EventSemaphore"):
                    si = i.sync_info
                    if si is not None and len(si.on_wait) > 1:
                        for w in si.on_wait[:-1]:
                            d = mybir.InstDrain(name=f"mmw_{k}", ins=[], outs=[])
                            k += 1
                            d.engine = eng
                            d.sync_info = mybir.SyncInfo(on_wait=[w], on_update=[])
                            out.append(d)
                        i.sync_info = mybir.SyncInfo(on_wait=[si.on_wait[-1]],
                                                     on_update=si.on_update)
                        changed = True
                out.append(i)
            if changed:
                bb.instructions = out


def _host_prep(inputs):
    """Slice/pad per-core tensors + reshape weights into device layouts."""
    f32 = np.float32
    bf = ml_dtypes.bfloat16
    x = [np.asarray(inputs[f"x{d}"], f32) for d in range(3)]
    pe = [np.asarray(inputs[f"pe{d}"], f32) for d in range(3)]
    bel = [np.asarray(inputs[f"belong{d}"]).astype(np.int64) for d in range(3)]
    adj = {0: np.asarray(inputs["adj00"], f32), 1: np.asarray(inputs["adj11"], f32),
           2: np.asarray(inputs["adj22"], f32)}
    b01 = np.asarray(inputs["b01"], f32)
    b12 = np.asarray(inputs["b12"], f32)

    # this build skips the all-zero biases / identity LN affine
    for nm in ("bf0", "bf1", "bf2", "bq", "bk", "bv", "bo", "be1", "be2",
               "bff1", "bff2", "bh1", "bh2", "bh3"):
        assert not np.any(np.asarray(inputs[nm])), f"nonzero bias {nm} unsupported"
    assert np.all(np.asarray(inputs["g1"]) == 1.0)
    assert np.all(np.asarray(inputs["g2"]) == 1.0)

    Wq = np.asarray(inputs["Wq"], f32)
    Wk = np.asarray(inputs["Wk"], f32)
    Wv = np.asarray(inputs["Wv"], f32)
    Wo = np.asarray(inputs["Wo"], f32)
    WqR_ = Wq.transpose(2, 0, 1, 3).reshape(H, L * 7, H)
    WkR_ = Wk.transpose(2, 0, 1, 3).reshape(H, L * 7, H)
    WoR_ = Wo.transpose(2, 0, 1, 3).reshape(H, L * 7, H)
    WqPad = np.zeros((H, L * 7, 3, 128), f32)
    WkPad = np.zeros((H, L * 7, 3, 128), f32)
    WoPad = np.zeros((128, L * 7, 2, H), f32)
    for hd in range(NH):
        t3, b3 = hd // 3, 32 * (hd % 3)
        WqPad[:, :, t3, b3:b3 + 8] = WqR_[:, :, 8 * hd:8 * hd + 8]
        WkPad[:, :, t3, b3:b3 + 8] = WkR_[:, :, 8 * hd:8 * hd + 8]
        if hd < 6:
            WoPad[b3:b3 + 8, :, hd // 3, :] = WoR_[8 * hd:8 * hd + 8]
        else:
            WoPad[96:104, :, hd - 6, :] = WoR_[8 * hd:8 * hd + 8]
    shared = {
        "WqP": WqPad, "WkP": WkPad, "WoP": WoPad,
        "WvR": np.ascontiguousarray(Wv.transpose(2, 0, 1, 3).reshape(H, L * 7, H)),
        "Wff1R": np.ascontiguousarray(
            np.asarray(inputs["Wff1"], f32).transpose(2, 0, 1, 3)
            .reshape(H, L * 3, 4 * H)),
        "Wff2R": np.ascontiguousarray(
            np.asarray(inputs["Wff2"], f32).reshape(L, 3, 2, 128, H)
            .transpose(3, 0, 1, 2, 4).reshape(128, L * 3 * 2, H)),
        "Wh1": np.ascontiguousarray(np.asarray(inputs["Wh1"], f32)),
        "Wh2": np.ascontiguousarray(np.asarray(inputs["Wh2"], f32)),
        "Wh3": np.ascontiguousarray(np.asarray(inputs["Wh3"], f32)),
    }
    for d in range(3):
        shared[f"Wf{d}"] = np.ascontiguousarray(np.asarray(inputs[f"Wf{d}"], f32))
        shared[f"Wp{d}"] = np.ascontiguousarray(np.asarray(inputs[f"Wp{d}"], f32))

    def chunked(m):  # [S, T] -> [128, nch, T]
        S, T = m.shape
        nch = S // 128
        return np.ascontiguousarray(m.reshape(nch, 128, T).transpose(1, 0, 2))

    in_maps = []
    for b in range(NCORES):
        rng = []
        for d in range(3):
            s = int(np.searchsorted(bel[d], b))
            e = int(np.searchsorted(bel[d], b + 1))
            assert e - s <= NPF[d], f"complex {b} dim {d} block {e - s} > {NPF[d]}"
            rng.append((s, e))
        cnt = sum(e - s for s, e in rng)
        m = dict(shared)
        for d in range(3):
            s, e = rng[d]
            n = e - s
            xt = np.zeros((FDIM, NPC[d]), f32)
            xt[:, :n] = x[d][s:e].T
            pt = np.zeros((PDIM, NPC[d]), f32)
            pt[:, :n] = pe[d][s:e].T
            vmf = np.zeros((NPC[d],), f32)
            vmf[:n] = 1.0 / max(cnt, 1)
            m[f"xT{d}"] = xt
            m[f"peT{d}"] = pt
            m[f"vm{d}"] = np.ascontiguousarray(
                vmf.reshape(NCH[d], 128).T.reshape(128, NCH[d]))
        blocks = {}
        for d in range(3):
            s, e = rng[d]
            blocks[(d, d)] = (adj[d][s:e, s:e] > 0).astype(f32)
        s0, e0 = rng[0]
        s1, e1 = rng[1]
        s2, e2 = rng[2]
        blk01 = (b01[s0:e0, s1:e1] > 0).astype(f32)   # [n0, n1]
        blk12 = (b12[s1:e1, s2:e2] > 0).astype(f32)   # [n1, n2]
        # mT[(sd,td)] is source-major [S, T]
        mts = {(0, 0): blocks[(0, 0)].T, (1, 1): blocks[(1, 1)].T,
               (2, 2): blocks[(2, 2)].T,
               (0, 1): blk01, (1, 0): blk01.T, (1, 2): blk12, (2, 1): blk12.T}
        for ii, (sd, td) in enumerate(INTER):
            buf = np.zeros((NPC[sd], NPF[td]), f32)
            mm = mts[(sd, td)]
            buf[: mm.shape[0], : mm.shape[1]] = mm
            buf[NPC[sd] - 1, :] = PHANTOM_EPS  # keeps every denominator > 0
            m[f"mT{ii}"] = chunked(buf).astype(bf)
        in_maps.append(m)
    return in_maps


def kernel(**inputs):
    global last_results
    if "nc" not in _PROGRAM_CACHE:
        nc = _build_program()
        _split_matmul_waits(nc)
        _PROGRAM_CACHE["nc"] = nc
    nc = _PROGRAM_CACHE["nc"]
    in_maps = _host_prep(inputs)
    res = run_bass_kernel_spmd(nc, in_maps, core_ids=list(range(NCORES)))
    last_results = res
    y = np.stack([np.asarray(res.results[b]["y"]).reshape(OUT) for b in range(NCORES)])
    return y.astype(np.float32)


# revision 21
# speedup vs baseline: 1.0759x; 1.0759x over previous
"""Trainium2 Bass kernel for nn_CellularTransformer.

Sharding: data-parallel over the B=8 complexes (one complex per NeuronCore).
belong* arrays are sorted, so each complex occupies a contiguous row range in
every cell dimension; the block-diagonal batch mask zeroes all cross-complex
attention, so each core only needs the diagonal blocks of the adjacency /
boundary matrices. Small weights are replicated.

Per-core padded sizes (multiples of 128 for partition chunking):
  dim0: 256 cells (actual max block 210), dim1: 512 (402), dim2: 256 (213).
Target-side free dims are 256/416/256 (>=256 keeps fp32r matmuls at full rate).

Head layout: PE operand/output partition bases must be in {0,32,64}, so the
8 heads live in 3 "slots" of a [128, 3, *] tile, 3 heads per slot at bases
0/32/64 (slot 2 holds 2 heads + a phantom filler).  QK^T is computed per head
in [source, target] orientation; exp (no max subtraction — scores are O(10))
runs on ScalarE into bf16; the 0/1 mask multiply runs on VectorE in bf16; the
AV matmul uses a 32-wide augmented V (8 v-cols, a ones column producing the
softmax denominator D at row 32*h3+8, zero padding) so each 32-row PSUM group
is fully written.  The last padded source row of every mask is 1e-30, so D is
never exactly 0 (no NaNs, and rows with no neighbors come out exactly 0 after
the mask, matching the reference's softmax(-1e30)*mask behavior).
"""

import numpy as np
import ml_dtypes

import concourse.bass as bass
import concourse.mybir as mybir
import concourse.tile as tile
from concourse.bass_utils import run_bass_kernel_spmd
from concourse.masks import make_identity

F32 = mybir.dt.float32
F32R = mybir.dt.float32r
BF16 = mybir.dt.bfloat16
AF = mybir.ActivationFunctionType
ALU = mybir.AluOpType

NCORES = 8
L, NH, DH, H, OUT = 2, 8, 8, 64, 10
FDIM, PDIM = 32, 16
INTER = [(0, 0), (0, 1), (1, 0), (1, 1), (1, 2), (2, 1), (2, 2)]
NCH = [2, 4, 2]                 # 128-partition source chunks per dim
NPC = [c * 128 for c in NCH]    # padded cell count (source side)
NPF = [256, 416, 256]           # padded cell count (target/free side)
SCALE = 1.0 / float(np.sqrt(DH))
TD_FIRST = {0: 0, 1: 1, 2: 4}
TD_LAST = {0: 2, 1: 5, 2: 6}
PHANTOM_EPS = 1e-30

_PROGRAM_CACHE = {}
last_results = None  # test.py introspection (exec time / profile)


def _build_program():
    nc = bass.Bass("TRN2", target_bir_lowering=False, debug=False, num_devices=NCORES)

    dt_in = {}

    def din(name, shape, dtype=F32):
        dt_in[name] = nc.dram_tensor(name, list(shape), dtype, kind="ExternalInput")

    for d in range(3):
        din(f"xT{d}", (FDIM, NPC[d]), F32R)
        din(f"peT{d}", (PDIM, NPC[d]), F32R)
        din(f"vm{d}", (128, NCH[d]))
    for ii, (sd, td) in enumerate(INTER):
        din(f"mT{ii}", (128, NCH[sd], NPF[td]), BF16)
    for d in range(3):
        din(f"Wf{d}", (FDIM, H), F32R)
        din(f"Wp{d}", (PDIM, H), F32R)
    din("WqP", (H, L * 7, 3, 128), F32R)
    din("WkP", (H, L * 7, 3, 128), F32R)
    din("WvR", (H, L * 7, H), F32R)
    din("WoP", (128, L * 7, 2, H), F32R)
    din("Wff1R", (H, L * 3, 4 * H), F32R)
    din("Wff2R", (128, L * 3 * 2, H), F32R)
    din("Wh1", (H, H))
    din("Wh2", (H, H))
    din("Wh3", (H, OUT))
    y_out = nc.dram_tensor("y", [1, OUT], F32, kind="ExternalOutput")

    from contextlib import ExitStack
    with tile.TileContext(nc) as tc, ExitStack() as es:
        const = es.enter_context(tc.tile_pool(name="const", bufs=1))
        state = es.enter_context(tc.tile_pool(name="state", bufs=1))
        work = es.enter_context(tc.tile_pool(name="work", bufs=3))
        pp_sc = es.enter_context(tc.tile_pool(name="pp_sc", bufs=2, space="PSUM"))
        pp_av = es.enter_context(tc.tile_pool(name="pp_av", bufs=1, space="PSUM"))
        pp_acc = es.enter_context(tc.tile_pool(name="pp_acc", bufs=1, space="PSUM"))
        pp_wk = es.enter_context(tc.tile_pool(name="pp_wk", bufs=2, space="PSUM"))

        ident = const.tile([128, 128], F32, tag="ident", name="ident")
        make_identity(nc, ident[:])

        def load(name, shape, tag, dtype=F32):
            t = const.tile(list(shape), dtype, tag=tag, name=tag)
            nc.sync.dma_start(out=t[:], in_=dt_in[name].ap())
            return t

        xT = [load(f"xT{d}", (FDIM, NPC[d]), f"xT{d}", F32R) for d in range(3)]
        peT = [load(f"peT{d}", (PDIM, NPC[d]), f"peT{d}", F32R) for d in range(3)]
        vm = [load(f"vm{d}", (128, NCH[d]), f"vm{d}") for d in range(3)]
        mT = [load(f"mT{ii}", (128, NCH[sd], NPF[td]), f"mT{ii}", BF16)
              for ii, (sd, td) in enumerate(INTER)]
        Wf = [load(f"Wf{d}", (FDIM, H), f"Wf{d}", F32R) for d in range(3)]
        Wp = [load(f"Wp{d}", (PDIM, H), f"Wp{d}", F32R) for d in range(3)]
        WqP = load("WqP", (H, L * 7, 3, 128), "WqP", F32R)
        WkP = load("WkP", (H, L * 7, 3, 128), "WkP", F32R)
        WvR = load("WvR", (H, L * 7, H), "WvR", F32R)
        WoP = load("WoP", (128, L * 7, 2, H), "WoP", F32R)
        Wff1 = load("Wff1R", (H, L * 3, 4 * H), "Wff1R", F32R)
        Wff2 = load("Wff2R", (128, L * 3 * 2, H), "Wff2R", F32R)
        Wh1 = load("Wh1", (H, H), "Wh1")
        Wh2 = load("Wh2", (H, H), "Wh2")
        Wh3 = load("Wh3", (H, OUT), "Wh3")

        # persistent state
        hT = [state.tile([H, NPC[d]], F32R, tag=f"hT{d}", name=f"hT{d}") for d in range(3)]
        h_ = [state.tile([128, NCH[d], H], F32, tag=f"h{d}", name=f"h{d}") for d in range(3)]
        h1 = [state.tile([128, NCH[d], H], F32, tag=f"h1{d}", name=f"h1{d}") for d in range(3)]
        hT1 = [state.tile([H, NPC[d]], F32R, tag=f"hT1{d}", name=f"hT1{d}") for d in range(3)]
        for d in range(3):
            nc.vector.memset(h_[d][:], 0.0)
            nc.vector.memset(h1[d][:], 0.0)
        # manually double-buffered augmented-V tiles: [128, chunk, head, 32]
        # col 8 = 1 (denominator source), cols 9..31 = 0 (full 32-row groups)
        vaug = [state.tile([128, 4, NH, 32], BF16, tag=f"vaug{j}", name=f"vaug{j}")
                for j in range(2)]
        vpat = state.tile([128, 32], F32, tag="vpat", name="vpat")
        nc.vector.memset(vpat[:], 0.0)
        nc.vector.memset(vpat[:, 8:9], 1.0)
        for j in range(2):
            nc.vector.tensor_copy(
                vaug[j][:],
                vpat[:].unsqueeze(1).unsqueeze(1).to_broadcast((128, 4, NH, 32)))
        eps = state.tile([128, 1], F32, tag="eps", name="eps")
        nc.vector.memset(eps[:], 1e-5)

        def r(ap):
            return ap.bitcast(F32R)

        # embed
        for d in range(3):
            ps = pp_wk.tile([128, 512], F32, tag="wk", name="wk")
            emb = ps[:H, : NPC[d]]
            nc.tensor.matmul(emb, r(Wf[d][:]), r(xT[d][:]), start=True, stop=False)
            nc.tensor.matmul(emb, r(Wp[d][:]), r(peT[d][:]), start=False, stop=True)
            nc.scalar.copy(out=hT[d][:], in_=emb)
            for c in range(NCH[d]):
                tp = pp_wk.tile([128, 512], F32, tag="wk", name="wk")
                nc.tensor.transpose(tp[:128, :H],
                                    hT[d][:, c * 128:(c + 1) * 128].bitcast(F32),
                                    ident[:H, :H])
                nc.scalar.copy(out=h_[d][:, c, :], in_=tp[:128, :H])

        shuf_mask = [8] * 32

        for l in range(L):
            acc02 = pp_acc.tile([H, 512], F32, tag="acc02", name="acc02")
            acc1 = pp_acc.tile([H, 512], F32, tag="acc1", name="acc1")
            accv = {0: acc02[:, 0:256], 1: acc1[:, 0:416], 2: acc02[:, 256:512]}
            for ii, (sd, td) in enumerate(INTER):
                li = l * 7 + ii
                T = NPF[td]
                nsc = NCH[sd]
                # qT/kT head-slot layout [128, 3, cells]; head hd -> slot
                # t3 = hd // 3 at partition base 32*(hd % 3)
                qT = work.tile([128, 3, 512], F32R, tag="qT", name="qT", bufs=2)
                kT = work.tile([128, 3, 512], F32R, tag="kT", name="kT", bufs=2)
                for t3 in range(3):
                    qps = pp_wk.tile([128, 512], F32, tag="wk", name="wk")
                    nc.tensor.matmul(qps[:, :T], r(WqP[:, li, t3, :]),
                                     r(hT[td][:, :T]), start=True, stop=True)
                    nc.vector.tensor_copy(qT[:, t3, :T], qps[:, :T])
                    kps = pp_wk.tile([128, 512], F32, tag="wk", name="wk")
                    nc.tensor.matmul(kps[:, : NPC[sd]], r(WkP[:, li, t3, :]),
                                     r(hT[sd][:]), start=True, stop=True)
                    nc.scalar.copy(out=kT[:, t3, : NPC[sd]], in_=kps[:, : NPC[sd]])
                vsb = vaug[ii % 2]
                for c in range(nsc):
                    vps = pp_wk.tile([128, 512], F32, tag="wk", name="wk")
                    nc.tensor.matmul(vps[:128, :H],
                                     r(hT[sd][:, c * 128:(c + 1) * 128]),
                                     r(WvR[:, li, :]), start=True, stop=True)
                    nc.vector.tensor_copy(
                        vsb[:, c, :, 0:8],
                        vps[:128, :H].rearrange("p (a b) -> p a b", a=NH))
                # attention: per head, single-bank score tiles (pipelined
                # with exp/mask), AV into a fully packed 2-slot psum: slot hf
                # holds heads 3*hf..3*hf+2 at bases 0/32/64 and head 6+hf at
                # base 96 (explicit tile_position, since implicit placement
                # rejects base 96).
                avp = pp_av.tile([128, 2, 512], F32, tag="av", name="av")
                for c in range(nsc):
                    for hd in range(NH):
                        t3, b3 = hd // 3, 32 * (hd % 3)
                        scp = pp_sc.tile([128, 512], F32, tag="sc", name="sc")
                        nc.tensor.matmul(
                            scp[:, :T],
                            r(kT[b3:b3 + 8, t3, c * 128:(c + 1) * 128]),
                            r(qT[b3:b3 + 8, t3, :T]),
                            start=True, stop=True)
                        et = work.tile([128, 512], BF16, tag="et", name="et", bufs=6)
                        nc.scalar.activation(out=et[:, :T], in_=scp[:, :T],
                                             func=AF.Exp, scale=SCALE)
                        pt = work.tile([128, 512], BF16, tag="pt", name="pt", bufs=6)
                        nc.vector.tensor_mul(pt[:, :T], et[:, :T], mT[ii][:, c, :])
                        if hd < 6:
                            hf, bo = hd // 3, 32 * (hd % 3)
                            tpos = None
                        else:
                            hf, bo = hd - 6, 96
                            tpos = (0, 96)
                        nc.tensor.matmul(
                            avp[bo:bo + 32, hf, :T],
                            vsb[:, c, hd, :],
                            pt[:, :T],
                            start=(c == 0), stop=(c == nsc - 1),
                            skip_group_check=True, tile_position=tpos)
                # normalize: o = av * shuffle(1/D); D at row base+8 of each
                # 32-group (phantom mask row keeps D >= 1e-30)
                rav = work.tile([128, 2, 512], F32, tag="rav", name="rav", bufs=2)
                nc.vector.stream_shuffle(out=rav[:, :, :T], in_=avp[:, :, :T],
                                         mask=shuf_mask)
                dbc = work.tile([128, 2, 512], F32, tag="dbc", name="dbc", bufs=2)
                nc.vector.reciprocal(out=dbc[:, :, :T], in_=rav[:, :, :T])
                onm = work.tile([128, 2, 512], F32R, tag="onm", name="onm", bufs=2)
                nc.vector.tensor_mul(onm[:, :, :T], avp[:, :, :T], dbc[:, :, :T])
                for hf in range(2):
                    nc.tensor.matmul(
                        accv[td], r(WoP[:, li, hf, :]),
                        r(onm[:, hf, :T]),
                        start=(ii == TD_FIRST[td] and hf == 0),
                        stop=(ii == TD_LAST[td] and hf == 1))

            # residual + LN1 + FFN + LN2 per dim
            for d in range(3):
                T = NPF[d]
                accs = work.tile([H, 512], F32, tag="accs", name="accs", bufs=2)
                nc.scalar.copy(out=accs[:, :T], in_=accv[d])
                for c in range((T + 127) // 128):
                    w = min(128, T - c * 128)
                    tp = pp_wk.tile([128, 512], F32, tag="wk", name="wk")
                    nc.tensor.transpose(tp[:w, :H], accs[:, c * 128:c * 128 + w],
                                        ident[:H, :H])
                    hs = work.tile([128, H], F32, tag="hs", name="hs")
                    nc.vector.tensor_add(hs[:w], h_[d][:w, c, :], tp[:w, :H])
                    st = work.tile([128, 6], F32, tag="st", name="st")
                    nc.vector.bn_stats(out=st[:w], in_=hs[:w])
                    mv = work.tile([128, 2], F32, tag="mv", name="mv")
                    nc.vector.bn_aggr(out=mv[:w], in_=st[:w])
                    sd_ = work.tile([128, 1], F32, tag="sd", name="sd")
                    nc.scalar.activation(out=sd_[:w], in_=mv[:w, 1:2], func=AF.Sqrt,
                                         bias=eps[:w])
                    nc.vector.reciprocal(out=sd_[:w], in_=sd_[:w])
                    nc.vector.tensor_scalar(out=h1[d][:w, c, :], in0=hs[:w],
                                            scalar1=mv[:w, 0:1], scalar2=sd_[:w],
                                            op0=ALU.subtract, op1=ALU.mult)
                    tp2 = pp_wk.tile([128, 512], F32, tag="wk", name="wk")
                    nc.tensor.transpose(tp2[:H, :w], h1[d][:w, c, :], ident[:w, :w])
                    nc.scalar.copy(out=hT1[d][:, c * 128:c * 128 + w], in_=tp2[:H, :w])
                f1 = work.tile([128, 2, 512], F32R, tag="f1", name="f1", bufs=2)
                for fc in range(2):
                    fps = pp_wk.tile([128, 512], F32, tag="wk", name="wk")
                    nc.tensor.matmul(fps[:, :T],
                                     r(Wff1[:, l * 3 + d, fc * 128:(fc + 1) * 128]),
                                     r(hT1[d][:, :T]), start=True, stop=True)
                    nc.scalar.activation(out=f1[:, fc, :T], in_=fps[:, :T],
                                         func=AF.Relu)
                for c in range((T + 127) // 128):
                    w = min(128, T - c * 128)
                    fp2 = pp_wk.tile([128, 512], F32, tag="wk", name="wk")
                    for fc in range(2):
                        nc.tensor.matmul(fp2[:w, :H],
                                         r(f1[:, fc, c * 128:c * 128 + w]),
                                         r(Wff2[:, (l * 3 + d) * 2 + fc, :]),
                                         start=(fc == 0), stop=(fc == 1))
                    hs2 = work.tile([128, H], F32, tag="hs2", name="hs2")
                    nc.vector.tensor_add(hs2[:w], h1[d][:w, c, :], fp2[:w, :H])
                    st2 = work.tile([128, 6], F32, tag="st2", name="st2")
                    nc.vector.bn_stats(out=st2[:w], in_=hs2[:w])
                    mv2 = work.tile([128, 2], F32, tag="mv2", name="mv2")
                    nc.vector.bn_aggr(out=mv2[:w], in_=st2[:w])
                    sd2 = work.tile([128, 1], F32, tag="sd2", name="sd2")
                    nc.scalar.activation(out=sd2[:w], in_=mv2[:w, 1:2], func=AF.Sqrt,
                                         bias=eps[:w])
                    nc.vector.reciprocal(out=sd2[:w], in_=sd2[:w])
                    nc.vector.tensor_scalar(out=h_[d][:w, c, :], in0=hs2[:w],
                                            scalar1=mv2[:w, 0:1], scalar2=sd2[:w],
                                            op0=ALU.subtract, op1=ALU.mult)
                    tp3 = pp_wk.tile([128, 512], F32, tag="wk", name="wk")
                    nc.tensor.transpose(tp3[:H, :w], h_[d][:w, c, :], ident[:w, :w])
                    nc.scalar.copy(out=hT[d][:, c * 128:c * 128 + w],
                                   in_=tp3[:H, :w])

        # pooling (masked mean; vm carries 1/count) + head MLP
        plp = pp_wk.tile([128, 512], F32, tag="wk", name="wk")
        first = True
        for d in range(3):
            for c in range(NCH[d]):
                last = (d == 2 and c == NCH[2] - 1)
                nc.tensor.matmul(plp[:1, :H], vm[d][:, c:c + 1], h_[d][:, c, :],
                                 start=first, stop=last)
                first = False
        pls = work.tile([1, H], F32, tag="pls", name="pls")
        nc.scalar.copy(out=pls[:], in_=plp[:1, :H])
        ptp = pp_wk.tile([128, 512], F32, tag="wk", name="wk")
        nc.tensor.transpose(ptp[:H, :1], pls[:], ident[:1, :1])
        plT = work.tile([H, 1], F32, tag="plT", name="plT")
        nc.scalar.copy(out=plT[:], in_=ptp[:H, :1])
        y1p = pp_wk.tile([128, 512], F32, tag="wk", name="wk")
        nc.tensor.matmul(y1p[:H, :1], Wh1[:], plT[:], start=True, stop=True)
        y1 = work.tile([H, 1], F32, tag="y1", name="y1")
        nc.scalar.activation(out=y1[:], in_=y1p[:H, :1], func=AF.Relu)
        y2p = pp_wk.tile([128, 512], F32, tag="wk", name="wk")
        nc.tensor.matmul(y2p[:H, :1], Wh2[:], y1[:], start=True, stop=True)
        y2 = work.tile([H, 1], F32, tag="y2", name="y2")
        nc.scalar.activation(out=y2[:], in_=y2p[:H, :1], func=AF.Relu)
        y3p = pp_wk.tile([128, 512], F32, tag="wk", name="wk")
        nc.tensor.matmul(y3p[:OUT, :1], Wh3[:], y2[:], start=True, stop=True)
        y3 = work.tile([OUT, 1], F32, tag="y3", name="y3")
        nc.scalar.copy(out=y3[:], in_=y3p[:OUT, :1])
        nc.sync.dma_start(out=y_out.ap().rearrange("a b -> b a"), in_=y3[:])

    return nc


_SPLIT_ENGINES = frozenset(("PE", "Activation", "DVE", "Pool", "SP"))


def _split_matmul_waits(nc):
    """TPB engine instructions carry at most one semaphore wait. Move all but
    one wait onto same-engine Drain instructions inserted just before."""
    k = 0
    for f in nc.m.functions:
        for bb in f.blocks:
            insts = bb.instructions
            out = []
            changed = False
            for i in insts:
                eng = getattr(i, "engine", None)
                if (eng is not None and eng.name in _SPLIT_ENGINES
                        and i.opcode != "EventSemaphore"):
                    si = i.sync_info
                    if si is not None and len(si.on_wait) > 1:
                        for w in si.on_wait[:-1]:
                            d = mybir.InstDrain(name=f"mmw_{k}", ins=[], outs=[])
                            k += 1
                            d.engine = eng
                            d.sync_info = mybir.SyncInfo(on_wait=[w], on_update=[])
                            out.append(d)
                        i.sync_info = mybir.SyncInfo(on_wait=[si.on_wait[-1]],
                                                     on_update=si.on_update)
                        changed = True
                out.append(i)
            if changed:
                bb.instructions = out


def _host_prep(inputs):
    """Slice/pad per-core tensors + reshape weights into device layouts."""
    f32 = np.float32
    bf = ml_dtypes.bfloat16
    x = [np.asarray(inputs[f"x{d}"], f32) for d in range(3)]
    pe = [np.asarray(inputs[f"pe{d}"], f32) for d in range(3)]
    bel = [np.asarray(inputs[f"belong{d}"]).astype(np.int64) for d in range(3)]
    adj = {0: np.asarray(inputs["adj00"], f32), 1: np.asarray(inputs["adj11"], f32),
           2: np.asarray(inputs["adj22"], f32)}
    b01 = np.asarray(inputs["b01"], f32)
    b12 = np.asarray(inputs["b12"], f32)

    # this build skips the all-zero biases / identity LN affine
    for nm in ("bf0", "bf1", "bf2", "bq", "bk", "bv", "bo", "be1", "be2",
               "bff1", "bff2", "bh1", "bh2", "bh3"):
        assert not np.any(np.asarray(inputs[nm])), f"nonzero bias {nm} unsupported"
    assert np.all(np.asarray(inputs["g1"]) == 1.0)
    assert np.all(np.asarray(inputs["g2"]) == 1.0)

    Wq = np.asarray(inputs["Wq"], f32)
    Wk = np.asarray(inputs["Wk"], f32)
    Wv = np.asarray(inputs["Wv"], f32)
    Wo = np.asarray(inputs["Wo"], f32)
    WqR_ = Wq.transpose(2, 0, 1, 3).reshape(H, L * 7, H)
    WkR_ = Wk.transpose(2, 0, 1, 3).reshape(H, L * 7, H)
    WoR_ = Wo.transpose(2, 0, 1, 3).reshape(H, L * 7, H)
    WqPad = np.zeros((H, L * 7, 3, 128), f32)
    WkPad = np.zeros((H, L * 7, 3, 128), f32)
    WoPad = np.zeros((128, L * 7, 2, H), f32)
    for hd in range(NH):
        t3, b3 = hd // 3, 32 * (hd % 3)
        WqPad[:, :, t3, b3:b3 + 8] = WqR_[:, :, 8 * hd:8 * hd + 8]
        WkPad[:, :, t3, b3:b3 + 8] = WkR_[:, :, 8 * hd:8 * hd + 8]
        if hd < 6:
            WoPad[b3:b3 + 8, :, hd // 3, :] = WoR_[8 * hd:8 * hd + 8]
        else:
            WoPad[96:104, :, hd - 6, :] = WoR_[8 * hd:8 * hd + 8]
    shared = {
        "WqP": WqPad, "WkP": WkPad, "WoP": WoPad,
        "WvR": np.ascontiguousarray(Wv.transpose(2, 0, 1, 3).reshape(H, L * 7, H)),
        "Wff1R": np.ascontiguousarray(
            np.asarray(inputs["Wff1"], f32).transpose(2, 0, 1, 3)
            .reshape(H, L * 3, 4 * H)),
        "Wff2R": np.ascontiguousarray(
            np.asarray(inputs["Wff2"], f32).reshape(L, 3, 2, 128, H)
            .transpose(3, 0, 1, 2, 4).reshape(128, L * 3 * 2, H)),
        "Wh1": np.ascontiguousarray(np.asarray(inputs["Wh1"], f32)),
        "Wh2": np.ascontiguousarray(np.asarray(inputs["Wh2"], f32)),
        "Wh3": np.ascontiguousarray(np.asarray(inputs["Wh3"], f32)),
    }
    for d in range(3):
        shared[f"Wf{d}"] = np.ascontiguousarray(np.asarray(inputs[f"Wf{d}"], f32))
        shared[f"Wp{d}"] = np.ascontiguousarray(np.asarray(inputs[f"Wp{d}"], f32))

    def chunked(m):  # [S, T] -> [128, nch, T]
        S, T = m.shape
        nch = S // 128
        return np.ascontiguousarray(m.reshape(nch, 128, T).transpose(1, 0, 2))

    in_maps = []
    for b in range(NCORES):
        rng = []
        for d in range(3):
            s = int(np.searchsorted(bel[d], b))
            e = int(np.searchsorted(bel[d], b + 1))
            assert e - s <= NPF[d], f"complex {b} dim {d} block {e - s} > {NPF[d]}"
            rng.append((s, e))
        cnt = sum(e - s for s, e in rng)
        m = dict(shared)
        for d in range(3):
            s, e = rng[d]
            n = e - s
            xt = np.zeros((FDIM, NPC[d]), f32)
            xt[:, :n] = x[d][s:e].T
            pt = np.zeros((PDIM, NPC[d]), f32)
            pt[:, :n] = pe[d][s:e].T
            vmf = np.zeros((NPC[d],), f32)
            vmf[:n] = 1.0 / max(cnt, 1)
            m[f"xT{d}"] = xt
            m[f"peT{d}"] = pt
            m[f"vm{d}"] = np.ascontiguousarray(
                vmf.reshape(NCH[d], 128).T.reshape(128, NCH[d]))
        blocks = {}
        for d in range(3):
            s, e = rng[d]
            blocks[(d, d)] = (adj[d][s:e, s:e] > 0).astype(f32)
        s0, e0 = rng[0]
        s1, e1 = rng[1]
        s2, e2 = rng[2]
        blk01 = (b01[s0:e0, s1:e1] > 0).astype(f32)   # [n0, n1]
        blk12 = (b12[s1:e1, s2:e2] > 0).astype(f32)   # [n1, n2]
        # mT[(sd,td)] is source-major [S, T]
        mts = {(0, 0): blocks[(0, 0)].T, (1, 1): blocks[(1, 1)].T,
               (2, 2): blocks[(2, 2)].T,
               (0, 1): blk01, (1, 0): blk01.T, (1, 2): blk12, (2, 1): blk12.T}
        for ii, (sd, td) in enumerate(INTER):
            buf = np.zeros((NPC[sd], NPF[td]), f32)
            mm = mts[(sd, td)]
            buf[: mm.shape[0], : mm.shape[1]] = mm
            buf[NPC[sd] - 1, :] = PHANTOM_EPS  # keeps every denominator > 0
            m[f"mT{ii}"] = chunked(buf).astype(bf)
        in_maps.append(m)
    return in_maps


def kernel(**inputs):
    global last_results
    if "nc" not in _PROGRAM_CACHE:
        nc = _build_program()
        _split_matmul_waits(nc)
        _PROGRAM_CACHE["nc"] = nc
    nc = _PROGRAM_CACHE["nc"]
    in_maps = _host_prep(inputs)
    res = run_bass_kernel_spmd(nc, in_maps, core_ids=list(range(NCORES)))
    last_results = res
    y = np.stack([np.asarray(res.results[b]["y"]).reshape(OUT) for b in range(NCORES)])
    return y.astype(np.float32)
